# revision 8
# baseline (speedup 1.0000x reference)
"""Trainium2 Bass kernel for nn_Conv_agg (edge-parallel GNN message passing).

Math (see reference):
    out[n] = sum_k ( sum_{e: src(e)=n} X[e,k] * h[tgt(e)] ) @ W[k] + bias

Structure exploited (asserted at runtime, guaranteed by setup_inputs):
  - src(e) = e // DEG exactly (each node emits DEG=16 consecutive edges)
  - edges/nodes of graph g are contiguous and tgt(e) stays inside graph g's
    100-node window -> sharding 125 graphs per core is a perfect partition
    (no cross-core edges, no collectives).

Gather-free formulation. Per graph g define the scatter matrix
    S_k[t, n] = sum_{e: src(e)=n} X[e,k] * onehot(tgt_local(e))[t]
so that  A_k = S_k.T? ... precisely  A_k[n, :] = sum_t S_k[t, n] * h_g[t, :]
and      out_g = sum_k A_k @ W_k + bias.

Device pipeline per chunk of 4096 edges (= 256 nodes = 32 PE blocks):
  1. DVE: OH[p, b, t] = (tgt_f16[p, b] == iota_row[t])      (one is_equal op)
  2. DVE: xall[p, b, j, k] = X_f16[p, b, k] * blockdiag[p, (j,k)]
  3. PE, per 128-edge block b:  psS[0:100, b, :] (16 cols = 8 nodes x 2 k)
         = OH_b[128e, 100t].T @ xall_b[128e, 16]     (scatter + segment sum)
  4. DVE: copy psS -> fp16 S.T flat buffer [100 t, 2*node + k]
  Per finished graph g (cols 200g..200g+200 ready):
  5. PE: A.T [128 c, 200] = h_g[100 t, 128 c].T @ S.T slice   (one matmul)
  6. DVE: copy A.T -> fp16
  7. PE: out_g[100 n, 128 o] = sum_k A_k.T.T @ W_k            (psum accum)
  8. DVE adds bias; batched DMA out.

All PE inputs fp16 (1 cycle/row vs 4 for fp32), PSUM accumulation fp32.
No DMA gather (the old kernel spent 1.69 ms of 1.74 ms in SWDGE
descriptor generation for per-edge 512B gathers).
"""

import numpy as np

B, NPG, DEG, K, CIN, COUT = 1000, 100, 16, 2, 128, 128
E = B * NPG * DEG            # 1,600,000 edges
NT = B * NPG                 # 100,000 nodes
NCORES = 8
NT_C = NT // NCORES          # 12,500 nodes / core
E_C = E // NCORES            # 200,000 edges / core
G_C = B // NCORES            # 125 graphs / core
EB = 32                      # 128-edge blocks per chunk
CHUNK_E = EB * 128           # 4096 edges / chunk
CHUNK_N = CHUNK_E // DEG     # 256 nodes / chunk
N_CHUNKS = -(-E_C // CHUNK_E)   # 49
E_PAD = N_CHUNKS * CHUNK_E   # 200,704
NB = E_PAD // 128            # 1568 blocks / core
FLAT_COLS = N_CHUNKS * 512   # 25,088 S.T columns (2*node + k)
TOUT = 5                     # graphs per output DMA slab

_module_cache = {}


def _patch_tile_drain():
    """This walrus build allows a single sync-wait per instruction; Tile's
    kernel-tail drain aggregates one wait per outstanding sem onto one
    InstDrain. Hoist extras onto dedicated sync nops (sequential on SP)."""
    import concourse.mybir as mybir
    from concourse.tile import TileContext
    from concourse.vector_clock import ScopedClock

    if getattr(TileContext, "_drain_patched", False):
        return

    def _drain_and_barrier(self, tick_clock, wait_clock):
        probe = self.nc.sync.nop(nofuse=True)
        wait_clock.add_sem_waits(probe.ins, ScopedClock({None: tick_clock.global_clock}))
        si = probe.ins.sync_info
        waits = list(si.on_wait) if si is not None and si.on_wait else []
        if si is not None and len(waits) > 1:
            si.on_wait = waits[:1]
            for w in waits[1:]:
                n = self.nc.sync.nop(nofuse=True)
                n.ins.sync_info = mybir.SyncInfo(on_wait=[w], on_update=[])
        self.nc.sync.drain()
        self.nc.all_engine_barrier()
        assert self.sems is not None
        popped = self.nc._tile_sem_poison_stack.pop()
        assert popped is self._sem_poison
        self.nc.clear_and_free_semaphores(list(self.sems.allocated().values()))
        self.nc.all_engine_barrier()

    TileContext._drain_and_barrier = _drain_and_barrier
    TileContext._drain_patched = True


def _build_module():
    import concourse.bacc as bacc
    import concourse.mybir as mybir
    from concourse.tile import TileContext

    _patch_tile_drain()
    f32 = mybir.dt.float32
    f16 = mybir.dt.float16

    nc = bacc.Bacc("TRN2", target_bir_lowering=False)
    # h relayouted on host to [100 part, 125 graph, 128 cin] f32
    h_t = nc.dram_tensor("h", [NPG, G_C * CIN], f32, kind="ExternalInput")
    x_t = nc.dram_tensor("x", [128, NB * K], f32, kind="ExternalInput")
    tgt_t = nc.dram_tensor("tgt", [128, NB], f16, kind="ExternalInput")
    # iota fully materialized [p, t, b] so the is_equal streams are all
    # innermost-contiguous -> DVE 2x_1P perf mode
    iota_t = nc.dram_tensor("iota", [128, NPG * EB], f16, kind="ExternalInput")
    mjk_t = nc.dram_tensor("mjk", [128, 8 * K], f16, kind="ExternalInput")
    w_t = nc.dram_tensor("w", [CIN, K * COUT], f32, kind="ExternalInput")
    bias_t = nc.dram_tensor("bias", [128, COUT], f32, kind="ExternalInput")
    out_t = nc.dram_tensor("out", [NT_C, COUT], f32, kind="ExternalOutput")

    with TileContext(nc) as tc:
        with (
            tc.tile_pool(name="consts", bufs=1) as cpool,
            tc.tile_pool(name="ohp", bufs=3) as ohpool,
            tc.tile_pool(name="xap", bufs=3) as xapool,
            tc.tile_pool(name="atp", bufs=3) as atpool,
            tc.tile_pool(name="osb", bufs=3) as opool,
            tc.tile_pool(name="psS", bufs=3, space="PSUM") as psumS,
            tc.tile_pool(name="psA", bufs=2, space="PSUM") as psumA,
            tc.tile_pool(name="psO", bufs=2, space="PSUM") as psumO,
        ):
            # --- constants / resident inputs ---
            iota_sb = cpool.tile([128, NPG, EB], f16)
            nc.sync.dma_start(iota_sb[:, :, :],
                              iota_t[:, :].rearrange("p (t b) -> p t b", b=EB))
            mjk_sb = cpool.tile([128, 8, K], f16)
            nc.sync.dma_start(mjk_sb[:, :, :],
                              mjk_t[:, :].rearrange("p (j k) -> p j k", k=K))
            bias_sb = cpool.tile([128, COUT], f32)
            nc.sync.dma_start(bias_sb[:, :], bias_t[:, :])
            w_sb = cpool.tile([128, K, COUT], f16)
            nc.gpsimd.dma_start(w_sb[:, :, :],
                                w_t[:, :].rearrange("c (k o) -> c k o", k=K))
            tgt_sb = cpool.tile([128, N_CHUNKS, EB], f16)
            nc.sync.dma_start(tgt_sb[:, :, :],
                              tgt_t[:, :].rearrange("p (c b) -> p c b", b=EB))
            # f32 -> f16 casts ride the SWDGE load
            x_sb = cpool.tile([128, N_CHUNKS, EB, K], f16)
            nc.gpsimd.dma_start(x_sb[:, :, :, :],
                                x_t[:, :].rearrange("p (c b k) -> p c b k",
                                                    b=EB, k=K))
            h_sb = cpool.tile([128, G_C, CIN], f16)
            nc.gpsimd.dma_start(h_sb[:NPG, :, :],
                                h_t[:, :].rearrange("p (g c) -> p g c", c=CIN))
            # S.T accumulator: col 2*node + k, partitions = local target id
            flat_sb = cpool.tile([128, FLAT_COLS], f16)

            gdone = 0
            for c in range(N_CHUNKS):
                # 1. one-hot of local target ids, all 32 blocks in one op
                # [p, t, b] layout: every stream innermost-contiguous (2x mode)
                oh = ohpool.tile([128, NPG, EB], f16)
                nc.vector.tensor_tensor(
                    oh[:, :, :],
                    tgt_sb[:, c, :].unsqueeze(1).broadcast_to([128, NPG, EB]),
                    iota_sb[:, :, :],
                    op=mybir.AluOpType.is_equal,
                )
                # 2. block-diag masked X: xall[p, b, j, k]
                xall = xapool.tile([128, EB, 8, K], f16)
                nc.vector.tensor_tensor(
                    xall[:, :, :, :],
                    x_sb[:, c, :, :].unsqueeze(2).broadcast_to([128, EB, 8, K]),
                    mjk_sb[:, :, :].unsqueeze(1).broadcast_to([128, EB, 8, K]),
                    op=mybir.AluOpType.mult,
                )
                # 3. scatter+segment-sum on PE: S.T chunk [100 t, 512]
                psS = psumS.tile([128, EB, 16], f32)
                for b in range(EB):
                    nc.tensor.matmul(
                        psS[:NPG, b, :],
                        oh[:, :, b],            # lhsT [128 e, 100 t]
                        xall[:, b, :, :],       # rhs  [128 e, 16]
                        start=True, stop=True,
                    )
                # 4. into the fp16 flat S.T buffer
                nc.any.tensor_copy(
                    flat_sb[:NPG, 512 * c:512 * (c + 1)],
                    psS[:NPG, :, :].rearrange("p b j -> p (b j)"))

                # 5-8. finish graphs fully covered by copied chunks
                while gdone < G_C and 200 * (gdone + 1) <= 512 * (c + 1):
                    g = gdone
                    psA = psumA.tile([128, NPG, K], f32)
                    nc.tensor.matmul(
                        psA[:, :, :],
                        h_sb[:NPG, g, :],                        # [100 t, 128 c]
                        flat_sb[:NPG, 200 * g:200 * (g + 1)]
                        .rearrange("t (n k) -> t n k", k=K),     # [100 t, 100, 2]
                        start=True, stop=True,
                    )
                    aT = atpool.tile([128, NPG, K], f16)
                    nc.any.tensor_copy(aT[:, :, :], psA[:, :, :])
                    psO = psumO.tile([128, COUT], f32)
                    for k in range(K):
                        nc.tensor.matmul(
                            psO[:NPG, :],
                            aT[:, :, k],                         # [128 c, 100 n]
                            w_sb[:, k, :],                       # [128 c, 128 o]
                            start=(k == 0), stop=(k == K - 1),
                        )
                    if g % TOUT == 0:
                        o_sb = opool.tile([128, TOUT, COUT], f32)
                    nc.vector.tensor_tensor(o_sb[:NPG, g % TOUT, :],
                                            psO[:NPG, :], bias_sb[:NPG, :],
                                            op=mybir.AluOpType.add)
                    if (g + 1) % TOUT == 0:
                        g0 = g + 1 - TOUT
                        nc.sync.dma_start(
                            out_t[:, :].rearrange("(g p) o -> p g o", p=NPG)
                            [:, g0:g0 + TOUT, :],
                            o_sb[:NPG, :, :])
                    gdone += 1
    nc.compile()
    return nc


def _get_module():
    if "nc" not in _module_cache:
        _module_cache["nc"] = _build_module()
    return _module_cache["nc"]


def _prep_inputs(h, X, tgt, weight, bias):
    """Host-side sharding/layout (index manipulation + pure movement only)."""
    # local target ids (0..99 within each graph), exact in fp16
    tgt_loc = tgt - (tgt // NPG) * NPG
    tgt_p = np.zeros((NCORES, E_PAD), np.float16)
    tgt_p[:, :E_C] = tgt_loc.reshape(NCORES, E_C).astype(np.float16)
    # [core, p, c*b] with e_local = c*4096 + b*128 + p
    tgt_arr = np.ascontiguousarray(
        tgt_p.reshape(NCORES, N_CHUNKS, EB, 128).transpose(0, 3, 1, 2)
    ).reshape(NCORES, 128, NB)

    xp = np.zeros((NCORES, E_PAD, K), np.float32)
    xp[:, :E_C] = X.reshape(NCORES, E_C, K)
    x_arr = np.ascontiguousarray(
        xp.reshape(NCORES, N_CHUNKS, EB, 128, K).transpose(0, 3, 1, 2, 4)
    ).reshape(NCORES, 128, NB * K)

    # h: [core, 100 part, 125 graph, 128 cin], partition-major contiguous
    h_arr = np.ascontiguousarray(
        h.reshape(NCORES, G_C, NPG, CIN).transpose(0, 2, 1, 3)
    ).reshape(NCORES, NPG, G_C * CIN)

    iota = np.tile(np.repeat(np.arange(NPG, dtype=np.float16), EB), (128, 1))
    mjk = np.zeros((128, 8, K), np.float16)
    for p in range(128):
        mjk[p, p // 16, :] = 1.0
    mjk = mjk.reshape(128, 8 * K)

    w_arr = np.ascontiguousarray(
        weight.transpose(1, 0, 2)).reshape(CIN, K * COUT).astype(np.float32)
    bias_rep = np.ascontiguousarray(
        np.broadcast_to(bias, (128, COUT))).astype(np.float32)
    return tgt_arr, x_arr, h_arr, iota, mjk, w_arr, bias_rep


def kernel(h, X, edge_index, node_index, batch_node, batch_edge, num_node,
           weight, bias):
    from concourse.bass_utils import run_bass_kernel_spmd

    h = np.asarray(h, np.float32)
    X = np.asarray(X, np.float32)
    edge_index = np.asarray(edge_index)
    weight = np.asarray(weight, np.float32)
    bias = np.asarray(bias, np.float32)

    src = np.asarray(edge_index[1])
    tgt = np.asarray(edge_index[2])
    # structural contract from setup_inputs (see module docstring)
    assert src.shape == (E,) and h.shape == (NT, CIN) and X.shape == (E, K)
    assert np.array_equal(src, np.arange(E, dtype=src.dtype) // DEG), \
        "edges not sorted as src=e//DEG"
    assert np.array_equal(tgt // NPG, src // NPG), "tgt escapes its graph"

    tgt_arr, x_arr, h_arr, iota, mjk, w_arr, bias_rep = _prep_inputs(
        h, X, tgt, weight, bias)

    nc = _get_module()
    in_maps = []
    for c in range(NCORES):
        in_maps.append({
            "h": h_arr[c],
            "x": x_arr[c],
            "tgt": tgt_arr[c],
            "iota": iota,
            "mjk": mjk,
            "w": w_arr,
            "bias": bias_rep,
        })
    res = run_bass_kernel_spmd(nc, in_maps, core_ids=list(range(NCORES)))
    out = np.concatenate([r["out"] for r in res.results], axis=0)
    return out


# revision 21
# speedup vs baseline: 1.0484x; 1.0484x over previous
"""Trainium2 Bass kernel for nn_Conv_agg (edge-parallel GNN message passing).

Math (see reference):
    out[n] = sum_k ( sum_{e: src(e)=n} X[e,k] * h[tgt(e)] ) @ W[k] + bias

Structure exploited (asserted at runtime, guaranteed by setup_inputs):
  - src(e) = e // DEG exactly (each node emits DEG=16 consecutive edges)
  - edges/nodes of graph g are contiguous and tgt(e) stays inside graph g's
    100-node window -> sharding 125 graphs per core is a perfect partition
    (no cross-core edges, no collectives).

Gather-free formulation. Per graph g define the scatter matrix
    S_k[t, n] = sum_{e: src(e)=n} X[e,k] * onehot(tgt_local(e))[t]
so that  A_k = S_k.T? ... precisely  A_k[n, :] = sum_t S_k[t, n] * h_g[t, :]
and      out_g = sum_k A_k @ W_k + bias.

Device pipeline per chunk of 4096 edges (= 256 nodes = 32 PE blocks):
  1. DVE: OH[p, b, t] = (tgt_f16[p, b] == iota_row[t])      (one is_equal op)
  2. DVE: xall[p, b, j, k] = X_f16[p, b, k] * blockdiag[p, (j,k)]
  3. PE, per 128-edge block b:  psS[0:100, b, :] (16 cols = 8 nodes x 2 k)
         = OH_b[128e, 100t].T @ xall_b[128e, 16]     (scatter + segment sum)
  4. DVE: copy psS -> fp16 S.T flat buffer [100 t, 2*node + k]
  Per finished graph g (cols 200g..200g+200 ready):
  5. PE: A.T [128 c, 200] = h_g[100 t, 128 c].T @ S.T slice   (one matmul)
  6. DVE: copy A.T -> fp16
  7. PE: out_g[100 n, 128 o] = sum_k A_k.T.T @ W_k            (psum accum)
  8. DVE adds bias; batched DMA out.

All PE inputs fp16 (1 cycle/row vs 4 for fp32), PSUM accumulation fp32.
No DMA gather (the old kernel spent 1.69 ms of 1.74 ms in SWDGE
descriptor generation for per-edge 512B gathers).
"""

import numpy as np

B, NPG, DEG, K, CIN, COUT = 1000, 100, 16, 2, 128, 128
E = B * NPG * DEG            # 1,600,000 edges
NT = B * NPG                 # 100,000 nodes
NCORES = 8
NT_C = NT // NCORES          # 12,500 nodes / core
E_C = E // NCORES            # 200,000 edges / core
G_C = B // NCORES            # 125 graphs / core
EB = 32                      # 128-edge blocks per chunk
CHUNK_E = EB * 128           # 4096 edges / chunk
CHUNK_N = CHUNK_E // DEG     # 256 nodes / chunk
N_CHUNKS = -(-E_C // CHUNK_E)   # 49
E_PAD = N_CHUNKS * CHUNK_E   # 200,704
NB = E_PAD // 128            # 1568 blocks / core
FLAT_COLS = N_CHUNKS * 512   # 25,088 S.T columns (2*node + k)
TOUT = 5                     # graphs per output DMA slab

_module_cache = {}


def _patch_tile_drain():
    """This walrus build allows a single sync-wait per instruction; Tile's
    kernel-tail drain aggregates one wait per outstanding sem onto one
    InstDrain. Hoist extras onto dedicated sync nops (sequential on SP)."""
    import concourse.mybir as mybir
    from concourse.tile import TileContext
    from concourse.vector_clock import ScopedClock

    if getattr(TileContext, "_drain_patched", False):
        return

    def _drain_and_barrier(self, tick_clock, wait_clock):
        probe = self.nc.sync.nop(nofuse=True)
        wait_clock.add_sem_waits(probe.ins, ScopedClock({None: tick_clock.global_clock}))
        si = probe.ins.sync_info
        waits = list(si.on_wait) if si is not None and si.on_wait else []
        if si is not None and len(waits) > 1:
            si.on_wait = waits[:1]
            for w in waits[1:]:
                n = self.nc.sync.nop(nofuse=True)
                n.ins.sync_info = mybir.SyncInfo(on_wait=[w], on_update=[])
        self.nc.sync.drain()
        self.nc.all_engine_barrier()
        assert self.sems is not None
        popped = self.nc._tile_sem_poison_stack.pop()
        assert popped is self._sem_poison
        self.nc.clear_and_free_semaphores(list(self.sems.allocated().values()))
        self.nc.all_engine_barrier()

    TileContext._drain_and_barrier = _drain_and_barrier
    TileContext._drain_patched = True


def _build_module():
    import concourse.bacc as bacc
    import concourse.mybir as mybir
    from concourse.tile import TileContext

    _patch_tile_drain()
    f32 = mybir.dt.float32
    f16 = mybir.dt.float16

    nc = bacc.Bacc("TRN2", target_bir_lowering=False)
    # h relayouted on host to [100 part, 125 graph, 128 cin] f32
    h_t = nc.dram_tensor("h", [NPG, G_C * CIN], f32, kind="ExternalInput")
    x_t = nc.dram_tensor("x", [128, NB * K], f32, kind="ExternalInput")
    tgt_t = nc.dram_tensor("tgt", [128, NB], f16, kind="ExternalInput")
    # iota materialized per-block [p, b, t] (t innermost) so the is_equal
    # streams are all innermost-contiguous -> DVE 2x_1P perf mode
    iota_t = nc.dram_tensor("iota", [128, EB * NPG], f16, kind="ExternalInput")
    mjk_t = nc.dram_tensor("mjk", [128, 8 * K], f16, kind="ExternalInput")
    w_t = nc.dram_tensor("w", [CIN, K * COUT], f32, kind="ExternalInput")
    bias_t = nc.dram_tensor("bias", [128, COUT], f32, kind="ExternalInput")
    out_t = nc.dram_tensor("out", [NT_C, COUT], f32, kind="ExternalOutput")

    with TileContext(nc) as tc:
        with (
            tc.tile_pool(name="consts", bufs=1) as cpool,
            tc.tile_pool(name="ohp", bufs=3) as ohpool,
            tc.tile_pool(name="trp", bufs=3) as trpool,
            tc.tile_pool(name="xap", bufs=3) as xapool,
            tc.tile_pool(name="atp", bufs=3) as atpool,
            tc.tile_pool(name="osb", bufs=3) as opool,
            tc.tile_pool(name="psS", bufs=3, space="PSUM") as psumS,
            tc.tile_pool(name="psA", bufs=2, space="PSUM") as psumA,
            tc.tile_pool(name="psO", bufs=2, space="PSUM") as psumO,
        ):
            # --- constants / resident inputs ---
            iota_sb = cpool.tile([128, EB, NPG], f16)
            nc.sync.dma_start(iota_sb[:, :, :],
                              iota_t[:, :].rearrange("p (b t) -> p b t", t=NPG))
            zero_sb = cpool.tile([128, 1], f16)
            nc.vector.tensor_copy(zero_sb[:, :], iota_sb[:, 0, 0].unsqueeze(1))
            mjk_sb = cpool.tile([128, 8, K], f16)
            nc.sync.dma_start(mjk_sb[:, :, :],
                              mjk_t[:, :].rearrange("p (j k) -> p j k", k=K))
            bias_sb = cpool.tile([128, COUT], f32)
            nc.sync.dma_start(bias_sb[:, :], bias_t[:, :])
            w_sb = cpool.tile([128, K, COUT], f16)
            nc.gpsimd.dma_start(w_sb[:, :, :],
                                w_t[:, :].rearrange("c (k o) -> c k o", k=K))
            tgt_sb = cpool.tile([128, N_CHUNKS, EB], f16)
            nc.sync.dma_start(tgt_sb[:, :, :],
                              tgt_t[:, :].rearrange("p (c b) -> p c b", b=EB))
            # f32 -> f16 casts ride the SWDGE load
            x_sb = cpool.tile([128, N_CHUNKS, EB, K], f16)
            nc.gpsimd.dma_start(x_sb[:, :, :, :],
                                x_t[:, :].rearrange("p (c b k) -> p c b k",
                                                    b=EB, k=K))
            h_sb = cpool.tile([128, G_C, CIN], f16)
            nc.gpsimd.dma_start(h_sb[:NPG, :, :],
                                h_t[:, :].rearrange("p (g c) -> p g c", c=CIN))
            # S.T accumulator: col 2*node + k, partitions = local target id
            flat_sb = cpool.tile([128, FLAT_COLS], f16)

            gdone = 0
            for c in range(N_CHUNKS):
                # 1. one-hot of local target ids, all 32 blocks in one op.
                # GpSimd (otherwise idle) materializes tgt replicated along t
                # so the DVE is_equal gets innermost-contiguous streams on all
                # operands -> 2x_1P perf mode. The oh tile is padded to 128
                # t-cols (100..127 stay stale garbage; the matching PSUM rows
                # are never read) so LDWEIGHTS sees a full 128-col fp16
                # stationary -> FWL + background pull-ahead.
                tgtr = trpool.tile([128, EB, NPG], f16)
                nc.gpsimd.tensor_tensor(
                    tgtr[:, :, :],
                    tgt_sb[:, c, :].unsqueeze(2).broadcast_to([128, EB, NPG]),
                    zero_sb[:, :].unsqueeze(1).broadcast_to([128, EB, NPG]),
                    op=mybir.AluOpType.add,
                )
                oh = ohpool.tile([128, EB, 128], f16)
                nc.vector.tensor_tensor(
                    oh[:, :, :NPG],
                    tgtr[:, :, :],
                    iota_sb[:, :, :],
                    op=mybir.AluOpType.is_equal,
                )
                # 2. block-diag masked X: xall[p, b, j, k]
                xall = xapool.tile([128, EB, 8, K], f16)
                nc.vector.tensor_tensor(
                    xall[:, :, :, :],
                    x_sb[:, c, :, :].unsqueeze(2).broadcast_to([128, EB, 8, K]),
                    mjk_sb[:, :, :].unsqueeze(1).broadcast_to([128, EB, 8, K]),
                    op=mybir.AluOpType.mult,
                )
                # 3. scatter+segment-sum on PE: S.T chunk [100 t, 512]
                psS = psumS.tile([128, EB, 16], f32)
                for b in range(EB):
                    nc.tensor.matmul(
                        psS[:, b, :],
                        oh[:, b, :],            # lhsT [128 e, 128 t-padded]
                        xall[:, b, :, :],       # rhs  [128 e, 16]
                        start=True, stop=True,
                    )
                # 4. into the fp16 flat S.T buffer
                nc.any.tensor_copy(
                    flat_sb[:NPG, 512 * c:512 * (c + 1)],
                    psS[:NPG, :, :].rearrange("p b j -> p (b j)"))

                # 5-8. finish graphs fully covered by chunks copied one
                # iteration ago (gives the ACT psum->flat copy a full chunk
                # of slack before the PE's in-order A-matmul waits on it)
                while gdone < G_C and 200 * (gdone + 1) <= 512 * c:
                    g = gdone
                    psA = psumA.tile([128, NPG, K], f32)
                    nc.tensor.matmul(
                        psA[:, :, :],
                        h_sb[:NPG, g, :],                        # [100 t, 128 c]
                        flat_sb[:NPG, 200 * g:200 * (g + 1)]
                        .rearrange("t (n k) -> t n k", k=K),     # [100 t, 100, 2]
                        start=True, stop=True,
                    )
                    # n padded to 128 stale cols (psO rows 100..127 unread)
                    # so the out-stage LDWEIGHTS also gets the FWL path
                    aT = atpool.tile([128, 128, K], f16)
                    nc.any.tensor_copy(aT[:, :NPG, :], psA[:, :, :])
                    psO = psumO.tile([128, COUT], f32)
                    for k in range(K):
                        nc.tensor.matmul(
                            psO[:, :],
                            aT[:, :, k],                         # [128 c, 128 n]
                            w_sb[:, k, :],                       # [128 c, 128 o]
                            start=(k == 0), stop=(k == K - 1),
                        )
                    if g % TOUT == 0:
                        o_sb = opool.tile([128, TOUT, COUT], f32)
                    nc.vector.tensor_tensor(o_sb[:NPG, g % TOUT, :],
                                            psO[:NPG, :], bias_sb[:NPG, :],
                                            op=mybir.AluOpType.add)
                    if (g + 1) % TOUT == 0:
                        g0 = g + 1 - TOUT
                        nc.sync.dma_start(
                            out_t[:, :].rearrange("(g p) o -> p g o", p=NPG)
                            [:, g0:g0 + TOUT, :],
                            o_sb[:NPG, :, :])
                    gdone += 1
            # graphs whose columns completed only with the last chunk
            while gdone < G_C:
                g = gdone
                psA = psumA.tile([128, NPG, K], f32)
                nc.tensor.matmul(
                    psA[:, :, :],
                    h_sb[:NPG, g, :],
                    flat_sb[:NPG, 200 * g:200 * (g + 1)]
                    .rearrange("t (n k) -> t n k", k=K),
                    start=True, stop=True,
                )
                aT = atpool.tile([128, 128, K], f16)
                nc.any.tensor_copy(aT[:, :NPG, :], psA[:, :, :])
                psO = psumO.tile([128, COUT], f32)
                for k in range(K):
                    nc.tensor.matmul(
                        psO[:, :],
                        aT[:, :, k],
                        w_sb[:, k, :],
                        start=(k == 0), stop=(k == K - 1),
                    )
                if g % TOUT == 0:
                    o_sb = opool.tile([128, TOUT, COUT], f32)
                nc.vector.tensor_tensor(o_sb[:NPG, g % TOUT, :],
                                        psO[:NPG, :], bias_sb[:NPG, :],
                                        op=mybir.AluOpType.add)
                if (g + 1) % TOUT == 0:
                    g0 = g + 1 - TOUT
                    nc.sync.dma_start(
                        out_t[:, :].rearrange("(g p) o -> p g o", p=NPG)
                        [:, g0:g0 + TOUT, :],
                        o_sb[:NPG, :, :])
                gdone += 1
    nc.compile()
    return nc


def _get_module():
    if "nc" not in _module_cache:
        _module_cache["nc"] = _build_module()
    return _module_cache["nc"]


def _prep_inputs(h, X, tgt, weight, bias):
    """Host-side sharding/layout (index manipulation + pure movement only)."""
    # local target ids (0..99 within each graph), exact in fp16
    tgt_loc = tgt - (tgt // NPG) * NPG
    tgt_p = np.zeros((NCORES, E_PAD), np.float16)
    tgt_p[:, :E_C] = tgt_loc.reshape(NCORES, E_C).astype(np.float16)
    # [core, p, c*b] with e_local = c*4096 + b*128 + p
    tgt_arr = np.ascontiguousarray(
        tgt_p.reshape(NCORES, N_CHUNKS, EB, 128).transpose(0, 3, 1, 2)
    ).reshape(NCORES, 128, NB)

    xp = np.zeros((NCORES, E_PAD, K), np.float32)
    xp[:, :E_C] = X.reshape(NCORES, E_C, K)
    x_arr = np.ascontiguousarray(
        xp.reshape(NCORES, N_CHUNKS, EB, 128, K).transpose(0, 3, 1, 2, 4)
    ).reshape(NCORES, 128, NB * K)

    # h: [core, 100 part, 125 graph, 128 cin], partition-major contiguous
    h_arr = np.ascontiguousarray(
        h.reshape(NCORES, G_C, NPG, CIN).transpose(0, 2, 1, 3)
    ).reshape(NCORES, NPG, G_C * CIN)

    iota = np.tile(np.tile(np.arange(NPG, dtype=np.float16), EB), (128, 1))
    mjk = np.zeros((128, 8, K), np.float16)
    for p in range(128):
        mjk[p, p // 16, :] = 1.0
    mjk = mjk.reshape(128, 8 * K)

    w_arr = np.ascontiguousarray(
        weight.transpose(1, 0, 2)).reshape(CIN, K * COUT).astype(np.float32)
    bias_rep = np.ascontiguousarray(
        np.broadcast_to(bias, (128, COUT))).astype(np.float32)
    return tgt_arr, x_arr, h_arr, iota, mjk, w_arr, bias_rep


def kernel(h, X, edge_index, node_index, batch_node, batch_edge, num_node,
           weight, bias):
    from concourse.bass_utils import run_bass_kernel_spmd

    h = np.asarray(h, np.float32)
    X = np.asarray(X, np.float32)
    edge_index = np.asarray(edge_index)
    weight = np.asarray(weight, np.float32)
    bias = np.asarray(bias, np.float32)

    src = np.asarray(edge_index[1])
    tgt = np.asarray(edge_index[2])
    # structural contract from setup_inputs (see module docstring)
    assert src.shape == (E,) and h.shape == (NT, CIN) and X.shape == (E, K)
    assert np.array_equal(src, np.arange(E, dtype=src.dtype) // DEG), \
        "edges not sorted as src=e//DEG"
    assert np.array_equal(tgt // NPG, src // NPG), "tgt escapes its graph"

    tgt_arr, x_arr, h_arr, iota, mjk, w_arr, bias_rep = _prep_inputs(
        h, X, tgt, weight, bias)

    nc = _get_module()
    in_maps = []
    for c in range(NCORES):
        in_maps.append({
            "h": h_arr[c],
            "x": x_arr[c],
            "tgt": tgt_arr[c],
            "iota": iota,
            "mjk": mjk,
            "w": w_arr,
            "bias": bias_rep,
        })
    res = run_bass_kernel_spmd(nc, in_maps, core_ids=list(range(NCORES)))
    out = np.concatenate([r["out"] for r in res.results], axis=0)
    return out


# revision 26
# speedup vs baseline: 1.6924x; 1.6143x over previous
"""Trainium2 Bass kernel for nn_Conv_agg (edge-parallel GNN message passing).

Math (see reference):
    out[n] = sum_k ( sum_{e: src(e)=n} X[e,k] * h[tgt(e)] ) @ W[k] + bias

Structure exploited (asserted at runtime, guaranteed by setup_inputs):
  - src(e) = e // DEG exactly (each node emits DEG=16 consecutive edges)
  - edges/nodes of graph g are contiguous and tgt(e) stays inside graph g's
    100-node window -> sharding 125 graphs per core is a perfect partition
    (no cross-core edges, no collectives).

Gather-free formulation. Per graph g define the scatter matrix
    S_k[t, n] = sum_{e: src(e)=n} X[e,k] * onehot(tgt_local(e))[t]
so that  A_k = S_k.T? ... precisely  A_k[n, :] = sum_t S_k[t, n] * h_g[t, :]
and      out_g = sum_k A_k @ W_k + bias.

Device pipeline per chunk of 4096 edges (= 256 nodes = 32 PE blocks):
  1. DVE: OH[p, b, t] = (tgt_f16[p, b] == iota_row[t])      (one is_equal op)
  2. DVE: xall[p, b, j, k] = X_f16[p, b, k] * blockdiag[p, (j,k)]
  3. PE, per 128-edge block b:  psS[0:100, b, :] (16 cols = 8 nodes x 2 k)
         = OH_b[128e, 100t].T @ xall_b[128e, 16]     (scatter + segment sum)
  4. DVE: copy psS -> fp16 S.T flat buffer [100 t, 2*node + k]
  Per finished graph g (cols 200g..200g+200 ready):
  5. PE: A.T [128 c, 200] = h_g[100 t, 128 c].T @ S.T slice   (one matmul)
  6. DVE: copy A.T -> fp16
  7. PE: out_g[100 n, 128 o] = sum_k A_k.T.T @ W_k            (psum accum)
  8. DVE adds bias; batched DMA out.

All PE inputs fp16 (1 cycle/row vs 4 for fp32), PSUM accumulation fp32.
No DMA gather (the old kernel spent 1.69 ms of 1.74 ms in SWDGE
descriptor generation for per-edge 512B gathers).
"""

import numpy as np

B, NPG, DEG, K, CIN, COUT = 1000, 100, 16, 2, 128, 128
E = B * NPG * DEG            # 1,600,000 edges
NT = B * NPG                 # 100,000 nodes
NCORES = 8
NT_C = NT // NCORES          # 12,500 nodes / core
E_C = E // NCORES            # 200,000 edges / core
G_C = B // NCORES            # 125 graphs / core
EB = 32                      # 128-edge blocks per chunk
CHUNK_E = EB * 128           # 4096 edges / chunk
CHUNK_N = CHUNK_E // DEG     # 256 nodes / chunk
N_CHUNKS = -(-E_C // CHUNK_E)   # 49
E_PAD = N_CHUNKS * CHUNK_E   # 200,704
NB = E_PAD // 128            # 1568 blocks / core
FLAT_COLS = N_CHUNKS * 512   # 25,088 S.T columns (2*node + k)
TOUT = 5                     # graphs per output DMA slab

_module_cache = {}


def _patch_tile_drain():
    """This walrus build allows a single sync-wait per instruction; Tile's
    kernel-tail drain aggregates one wait per outstanding sem onto one
    InstDrain. Hoist extras onto dedicated sync nops (sequential on SP)."""
    import concourse.mybir as mybir
    from concourse.tile import TileContext
    from concourse.vector_clock import ScopedClock

    if getattr(TileContext, "_drain_patched", False):
        return

    def _drain_and_barrier(self, tick_clock, wait_clock):
        probe = self.nc.sync.nop(nofuse=True)
        wait_clock.add_sem_waits(probe.ins, ScopedClock({None: tick_clock.global_clock}))
        si = probe.ins.sync_info
        waits = list(si.on_wait) if si is not None and si.on_wait else []
        if si is not None and len(waits) > 1:
            si.on_wait = waits[:1]
            for w in waits[1:]:
                n = self.nc.sync.nop(nofuse=True)
                n.ins.sync_info = mybir.SyncInfo(on_wait=[w], on_update=[])
        self.nc.sync.drain()
        self.nc.all_engine_barrier()
        assert self.sems is not None
        popped = self.nc._tile_sem_poison_stack.pop()
        assert popped is self._sem_poison
        self.nc.clear_and_free_semaphores(list(self.sems.allocated().values()))
        self.nc.all_engine_barrier()

    TileContext._drain_and_barrier = _drain_and_barrier
    TileContext._drain_patched = True


def _build_module():
    import concourse.bacc as bacc
    import concourse.mybir as mybir
    from concourse.tile import TileContext

    _patch_tile_drain()
    f32 = mybir.dt.float32
    f16 = mybir.dt.float16

    nc = bacc.Bacc("TRN2", target_bir_lowering=False)
    # h relayouted on host to [100 part, 125 graph, 128 cin] f32
    h_t = nc.dram_tensor("h", [NPG, G_C * CIN], f32, kind="ExternalInput")
    x_t = nc.dram_tensor("x", [128, NB * K], f32, kind="ExternalInput")
    tgt_t = nc.dram_tensor("tgt", [128, NB], f16, kind="ExternalInput")
    iota_t = nc.dram_tensor("iota", [128, NPG], f16, kind="ExternalInput")
    mjk_t = nc.dram_tensor("mjk", [128, 8 * K], f16, kind="ExternalInput")
    w_t = nc.dram_tensor("w", [CIN, K * COUT], f32, kind="ExternalInput")
    bias_t = nc.dram_tensor("bias", [128, COUT], f32, kind="ExternalInput")
    out_t = nc.dram_tensor("out", [NT_C, COUT], f32, kind="ExternalOutput")

    with TileContext(nc) as tc:
        with (
            tc.tile_pool(name="consts", bufs=1) as cpool,
            tc.tile_pool(name="ohp", bufs=3) as ohpool,
            tc.tile_pool(name="xap", bufs=3) as xapool,
            tc.tile_pool(name="atp", bufs=3) as atpool,
            tc.tile_pool(name="osb", bufs=3) as opool,
            tc.tile_pool(name="psS", bufs=3, space="PSUM") as psumS,
            tc.tile_pool(name="psA", bufs=2, space="PSUM") as psumA,
            tc.tile_pool(name="psO", bufs=2, space="PSUM") as psumO,
        ):
            # --- constants / resident inputs ---
            iota_sb = cpool.tile([128, NPG], f16)
            nc.sync.dma_start(iota_sb[:, :], iota_t[:, :])
            mjk_sb = cpool.tile([128, 8, K], f16)
            nc.sync.dma_start(mjk_sb[:, :, :],
                              mjk_t[:, :].rearrange("p (j k) -> p j k", k=K))
            bias_sb = cpool.tile([128, COUT], f32)
            nc.sync.dma_start(bias_sb[:, :], bias_t[:, :])
            w_sb = cpool.tile([128, K, COUT], f16)
            nc.gpsimd.dma_start(w_sb[:, :, :],
                                w_t[:, :].rearrange("c (k o) -> c k o", k=K))
            tgt_sb = cpool.tile([128, N_CHUNKS, EB], f16)
            nc.sync.dma_start(tgt_sb[:, :, :],
                              tgt_t[:, :].rearrange("p (c b) -> p c b", b=EB))
            # f32 -> f16 casts ride the SWDGE load
            x_sb = cpool.tile([128, N_CHUNKS, EB, K], f16)
            nc.gpsimd.dma_start(x_sb[:, :, :, :],
                                x_t[:, :].rearrange("p (c b k) -> p c b k",
                                                    b=EB, k=K))
            h_sb = cpool.tile([128, G_C, CIN], f16)
            nc.gpsimd.dma_start(h_sb[:NPG, :, :],
                                h_t[:, :].rearrange("p (g c) -> p g c", c=CIN))
            # S.T accumulator: col 2*node + k, partitions = local target id
            flat_sb = cpool.tile([128, FLAT_COLS], f16)

            gdone = 0
            for c in range(N_CHUNKS):
                # 1. one-hot of local target ids, all 32 blocks in one op.
                # The oh tile is padded to 128 t-cols (100..127 stay stale
                # garbage; the matching PSUM rows are never read) so
                # LDWEIGHTS sees a full 128-col fp16 stationary -> FWL +
                # background-buffer pull-ahead.
                oh = ohpool.tile([128, EB, 128], f16)
                nc.vector.tensor_tensor(
                    oh[:, :, :NPG],
                    tgt_sb[:, c, :].unsqueeze(2).broadcast_to([128, EB, NPG]),
                    iota_sb[:, :].unsqueeze(1).broadcast_to([128, EB, NPG]),
                    op=mybir.AluOpType.is_equal,
                )
                # 2. block-diag masked X: xall[p, b, j, k]
                xall = xapool.tile([128, EB, 8, K], f16)
                nc.vector.tensor_tensor(
                    xall[:, :, :, :],
                    x_sb[:, c, :, :].unsqueeze(2).broadcast_to([128, EB, 8, K]),
                    mjk_sb[:, :, :].unsqueeze(1).broadcast_to([128, EB, 8, K]),
                    op=mybir.AluOpType.mult,
                )
                # 3. scatter+segment-sum on PE: S.T chunk [100 t, 512]
                psS = psumS.tile([128, EB, 16], f32)
                for b in range(EB):
                    nc.tensor.matmul(
                        psS[:, b, :],
                        oh[:, b, :],            # lhsT [128 e, 128 t-padded]
                        xall[:, b, :, :],       # rhs  [128 e, 16]
                        start=True, stop=True,
                    )
                # 4. into the fp16 flat S.T buffer
                nc.any.tensor_copy(
                    flat_sb[:NPG, 512 * c:512 * (c + 1)],
                    psS[:NPG, :, :].rearrange("p b j -> p (b j)"))

                # 5-8. finish graphs fully covered by chunks copied one
                # iteration ago (gives the ACT psum->flat copy a full chunk
                # of slack before the PE's in-order A-matmul waits on it)
                while gdone < G_C and 200 * (gdone + 1) <= 512 * c:
                    g = gdone
                    psA = psumA.tile([128, NPG, K], f32)
                    nc.tensor.matmul(
                        psA[:, :, :],
                        h_sb[:NPG, g, :],                        # [100 t, 128 c]
                        flat_sb[:NPG, 200 * g:200 * (g + 1)]
                        .rearrange("t (n k) -> t n k", k=K),     # [100 t, 100, 2]
                        start=True, stop=True,
                    )
                    # n padded to 128 stale cols (psO rows 100..127 unread)
                    # so the out-stage LDWEIGHTS also gets the FWL path
                    aT = atpool.tile([128, 128, K], f16)
                    nc.any.tensor_copy(aT[:, :NPG, :], psA[:, :, :])
                    psO = psumO.tile([128, COUT], f32)
                    for k in range(K):
                        nc.tensor.matmul(
                            psO[:, :],
                            aT[:, :, k],                         # [128 c, 128 n]
                            w_sb[:, k, :],                       # [128 c, 128 o]
                            start=(k == 0), stop=(k == K - 1),
                        )
                    if g % TOUT == 0:
                        o_sb = opool.tile([128, TOUT, COUT], f32)
                    nc.vector.tensor_tensor(o_sb[:NPG, g % TOUT, :],
                                            psO[:NPG, :], bias_sb[:NPG, :],
                                            op=mybir.AluOpType.add)
                    if (g + 1) % TOUT == 0:
                        g0 = g + 1 - TOUT
                        nc.sync.dma_start(
                            out_t[:, :].rearrange("(g p) o -> p g o", p=NPG)
                            [:, g0:g0 + TOUT, :],
                            o_sb[:NPG, :, :])
                    gdone += 1
            # graphs whose columns completed only with the last chunk
            while gdone < G_C:
                g = gdone
                psA = psumA.tile([128, NPG, K], f32)
                nc.tensor.matmul(
                    psA[:, :, :],
                    h_sb[:NPG, g, :],
                    flat_sb[:NPG, 200 * g:200 * (g + 1)]
                    .rearrange("t (n k) -> t n k", k=K),
                    start=True, stop=True,
                )
                aT = atpool.tile([128, 128, K], f16)
                nc.any.tensor_copy(aT[:, :NPG, :], psA[:, :, :])
                psO = psumO.tile([128, COUT], f32)
                for k in range(K):
                    nc.tensor.matmul(
                        psO[:, :],
                        aT[:, :, k],
                        w_sb[:, k, :],
                        start=(k == 0), stop=(k == K - 1),
                    )
                if g % TOUT == 0:
                    o_sb = opool.tile([128, TOUT, COUT], f32)
                nc.vector.tensor_tensor(o_sb[:NPG, g % TOUT, :],
                                        psO[:NPG, :], bias_sb[:NPG, :],
                                        op=mybir.AluOpType.add)
                if (g + 1) % TOUT == 0:
                    g0 = g + 1 - TOUT
                    nc.sync.dma_start(
                        out_t[:, :].rearrange("(g p) o -> p g o", p=NPG)
                        [:, g0:g0 + TOUT, :],
                        o_sb[:NPG, :, :])
                gdone += 1
    nc.compile()
    return nc


def _get_module():
    if "nc" not in _module_cache:
        _module_cache["nc"] = _build_module()
    return _module_cache["nc"]


def _prep_inputs(h, X, tgt, weight, bias):
    """Host-side sharding/layout (index manipulation + pure movement only)."""
    # local target ids (0..99 within each graph), exact in fp16
    tgt_loc = tgt - (tgt // NPG) * NPG
    tgt_p = np.zeros((NCORES, E_PAD), np.float16)
    tgt_p[:, :E_C] = tgt_loc.reshape(NCORES, E_C).astype(np.float16)
    # [core, p, c*b] with e_local = c*4096 + b*128 + p
    tgt_arr = np.ascontiguousarray(
        tgt_p.reshape(NCORES, N_CHUNKS, EB, 128).transpose(0, 3, 1, 2)
    ).reshape(NCORES, 128, NB)

    xp = np.zeros((NCORES, E_PAD, K), np.float32)
    xp[:, :E_C] = X.reshape(NCORES, E_C, K)
    x_arr = np.ascontiguousarray(
        xp.reshape(NCORES, N_CHUNKS, EB, 128, K).transpose(0, 3, 1, 2, 4)
    ).reshape(NCORES, 128, NB * K)

    # h: [core, 100 part, 125 graph, 128 cin], partition-major contiguous
    h_arr = np.ascontiguousarray(
        h.reshape(NCORES, G_C, NPG, CIN).transpose(0, 2, 1, 3)
    ).reshape(NCORES, NPG, G_C * CIN)

    iota = np.tile(np.arange(NPG, dtype=np.float16), (128, 1))
    mjk = np.zeros((128, 8, K), np.float16)
    for p in range(128):
        mjk[p, p // 16, :] = 1.0
    mjk = mjk.reshape(128, 8 * K)

    w_arr = np.ascontiguousarray(
        weight.transpose(1, 0, 2)).reshape(CIN, K * COUT).astype(np.float32)
    bias_rep = np.ascontiguousarray(
        np.broadcast_to(bias, (128, COUT))).astype(np.float32)
    return tgt_arr, x_arr, h_arr, iota, mjk, w_arr, bias_rep


def kernel(h, X, edge_index, node_index, batch_node, batch_edge, num_node,
           weight, bias):
    from concourse.bass_utils import run_bass_kernel_spmd

    h = np.asarray(h, np.float32)
    X = np.asarray(X, np.float32)
    edge_index = np.asarray(edge_index)
    weight = np.asarray(weight, np.float32)
    bias = np.asarray(bias, np.float32)

    src = np.asarray(edge_index[1])
    tgt = np.asarray(edge_index[2])
    # structural contract from setup_inputs (see module docstring)
    assert src.shape == (E,) and h.shape == (NT, CIN) and X.shape == (E, K)
    assert np.array_equal(src, np.arange(E, dtype=src.dtype) // DEG), \
        "edges not sorted as src=e//DEG"
    assert np.array_equal(tgt // NPG, src // NPG), "tgt escapes its graph"

    tgt_arr, x_arr, h_arr, iota, mjk, w_arr, bias_rep = _prep_inputs(
        h, X, tgt, weight, bias)

    nc = _get_module()
    in_maps = []
    for c in range(NCORES):
        in_maps.append({
            "h": h_arr[c],
            "x": x_arr[c],
            "tgt": tgt_arr[c],
            "iota": iota,
            "mjk": mjk,
            "w": w_arr,
            "bias": bias_rep,
        })
    res = run_bass_kernel_spmd(nc, in_maps, core_ids=list(range(NCORES)))
    out = np.concatenate([r["out"] for r in res.results], axis=0)
    return out
